# revision 2
# baseline (speedup 1.0000x reference)
"""GQA kernel for trn2, 8 NeuronCores, tensor-parallel over heads.

Sharding: 4 q heads + 1 kv head per core (column-split Wq/Wk/Wv, row-split
Wo), partial outputs summed on host. bf16 matmuls, fp32 PSUM.

Design (vs naive phase-serial version):
  - software-pipelined over 4 qs-chunks of 512: per chunk, scores+exp are
    emitted first (feed the ACT engine asap), then the NEXT chunk's
    projections (their long DVE chain overlaps this chunk's softmax),
    then AV/den/normalization, then the output projection.
  - x/weight DMAs chunked and interleaved so the first projection matmuls
    start as soon as (wq[k], xt[k] chunk 0) land.
  - score matmuls for a head PAIR run concurrently via 64x128 row tiling
    (heads at SBUF partitions 0-63/64-127, kro duplicated on both halves,
    outputs in the two banks of one [128,2,512] PSUM tile); one EXP
    instruction covers both heads via a strided AP.
  - AV col-packed via 128x64 tiling (shared vr lhsT for tiles (0,0)/(0,64));
    softmax denominators via ones-column lhsT matmuls in the same mode,
    accumulated per head in rows 0/64 of one PSUM bank.
  - all rsqrts (QK-norm) computed on DVE with a quake-seed + Newton step
    (int ALU on bitcast fp32), so the ACT table set never leaves
    exp_and_others: zero table reloads in steady state.
  - q-norm gains broadcast via selector matmuls; softmax 1/den via DVE
    reciprocal_approx_fast + selector matmul; causal tri masking on GpSimd.
"""

import sys
import types
import numpy as np
import ml_dtypes

for _p in ("/opt/trn_rl_repo",):
    if _p not in sys.path:
        sys.path.append(_p)

SEQ = 2048
DIM = 2048
HD = 64
NCORES = 8
EPS = 1e-6
THETA = 10000.0
NCH = 4          # qs chunks
CW = 512         # chunk width

_CACHE = {}


def _maybe_enable_ldw_opt():
    """Experiment: walrus is invoked with --enable-ldw-opt=false by default;
    flip it via env BASS_LDW_OPT=1 to measure the LDWEIGHTS pipelining win."""
    import os
    if os.environ.get("BASS_LDW_OPT") != "1":
        return
    from concourse import bass_utils as bu
    if getattr(bu, "_ldw_patched", False):
        return
    orig = bu.run_command

    def run_command(cmd, *a, **kw):
        if isinstance(cmd, list):
            cmd = ["--enable-ldw-opt=true" if c == "--enable-ldw-opt=false"
                   else c for c in cmd]
        return orig(cmd, *a, **kw)

    bu.run_command = run_command
    bu._ldw_patched = True


def _ensure_ntff_hook():
    if "antenv.axon_hooks" in sys.modules:
        return
    try:
        import antenv
        m = types.ModuleType("antenv.axon_hooks")
        hook = [None]
        m.set_axon_ntff_profile_hook = lambda h: hook.__setitem__(0, h)
        m.get_axon_ntff_profile_hook = lambda: hook[0]
        sys.modules["antenv.axon_hooks"] = m
        antenv.axon_hooks = m
        from trn_agent_boot.trn_boot import _ntff_profile_via_ctypes
        m.set_axon_ntff_profile_hook(
            _ntff_profile_via_ctypes("/opt/axon/libaxon_pjrt.so"))
    except Exception:
        pass


def _build_nc():
    import concourse.mybir as mybir
    import concourse.tile as tile
    from concourse import bacc

    f32, f16 = mybir.dt.float32, mybir.dt.bfloat16
    EXPF = mybir.ActivationFunctionType.Exp
    SQF = mybir.ActivationFunctionType.Square

    nc = bacc.Bacc("TRN2", target_bir_lowering=False, debug=False,
                   num_devices=NCORES)

    d_xt = nc.dram_tensor("xt", (16, 128, SEQ), f16, kind="ExternalInput")
    d_wq = nc.dram_tensor("wq", (16, 128, 256), f16, kind="ExternalInput")
    d_wkv = nc.dram_tensor("wkv", (16, 128, 128), f16, kind="ExternalInput")
    d_wo = nc.dram_tensor("wo", (2, 128, DIM), f16, kind="ExternalInput")
    d_cosq = nc.dram_tensor("cosq", (128, SEQ), f16, kind="ExternalInput")
    d_sinq = nc.dram_tensor("sinq", (128, SEQ), f16, kind="ExternalInput")
    d_cosk = nc.dram_tensor("cosk", (64, SEQ), f16, kind="ExternalInput")
    d_sink = nc.dram_tensor("sink", (64, SEQ), f16, kind="ExternalInput")
    d_tri = nc.dram_tensor("tri", (128, 128), f16, kind="ExternalInput")
    d_idn = nc.dram_tensor("idn", (64, 64), f16, kind="ExternalInput")
    d_ob = nc.dram_tensor("ob", (128, 128), f16, kind="ExternalInput")
    d_selq0 = nc.dram_tensor("selq0", (128, 128), f16, kind="ExternalInput")
    d_selq1 = nc.dram_tensor("selq1", (128, 128), f16, kind="ExternalInput")
    d_selk = nc.dram_tensor("selk", (128, 128), f16, kind="ExternalInput")
    d_seld = nc.dram_tensor("seld", (128, 128), f16, kind="ExternalInput")
    d_one64 = nc.dram_tensor("one64", (128, 64), f16, kind="ExternalInput")
    d_y = nc.dram_tensor("y", (16, 128, DIM), f16, kind="ExternalOutput")

    with tile.TileContext(nc) as tc:
        from contextlib import ExitStack
        with ExitStack() as ctx:
            kconst = ctx.enter_context(tc.tile_pool(name="kconst", bufs=1))
            xpool = ctx.enter_context(tc.tile_pool(name="xp", bufs=1))
            work = ctx.enter_context(tc.tile_pool(name="work", bufs=1))
            expool = ctx.enter_context(tc.tile_pool(name="ep", bufs=16))
            scp = ctx.enter_context(
                tc.tile_pool(name="scp", bufs=1, space="PSUM"))
            otp = ctx.enter_context(
                tc.tile_pool(name="otp", bufs=2, space="PSUM"))
            denp = ctx.enter_context(
                tc.tile_pool(name="denp", bufs=2, space="PSUM"))
            aux = ctx.enter_context(
                tc.tile_pool(name="aux", bufs=2, space="PSUM"))

            # ---- small consts first (cheap DMAs) --------------------------
            tri_sb = kconst.tile([128, 128], f16, tag="tri")
            nc.sync.dma_start(out=tri_sb, in_=d_tri[:, :])
            idn_sb = kconst.tile([64, 64], f16, tag="idn")
            nc.sync.dma_start(out=idn_sb, in_=d_idn[:, :])
            ob_sb = kconst.tile([128, 128], f16, tag="ob")
            nc.sync.dma_start(out=ob_sb, in_=d_ob[:, :])
            selq_sb = [kconst.tile([128, 128], f16, tag=f"selq{g}",
                                   name=f"selq{g}") for g in range(2)]
            nc.sync.dma_start(out=selq_sb[0], in_=d_selq0[:, :])
            nc.sync.dma_start(out=selq_sb[1], in_=d_selq1[:, :])
            selk_sb = kconst.tile([128, 128], f16, tag="selk")
            nc.sync.dma_start(out=selk_sb, in_=d_selk[:, :])
            seld_sb = kconst.tile([128, 128], f16, tag="seld")
            nc.sync.dma_start(out=seld_sb, in_=d_seld[:, :])
            one64_sb = kconst.tile([128, 64], f16, tag="one64")
            nc.sync.dma_start(out=one64_sb, in_=d_one64[:, :])

            # rsqrt scratch (rows 0-4 live; rest zeroed once)
            nrm = kconst.tile([128, CW], f32, tag="nrm")
            nc.vector.memset(nrm, 1.0)
            nrmb = kconst.tile([128, CW], f16, tag="nrmb")
            nc.vector.memset(nrmb, 0.0)
            y0t = kconst.tile([72, CW], f32, tag="y0t")
            nrt = kconst.tile([72, CW], f32, tag="nrt")
            rdenb = kconst.tile([128, CW], f16, tag="rdenb")
            nc.vector.memset(rdenb, 0.0)

            # weights + x chunk 0 interleaved per k so the first projection
            # matmuls can start as soon as (wq[k], xt[k]) land
            wq_sb = kconst.tile([128, 16, 256], f16, tag="wq")
            wkv_sb = kconst.tile([128, 16, 128], f16, tag="wkv")
            xts = [xpool.tile([128, SEQ], f16, tag=f"xt{i}", name=f"xt{i}")
                   for i in range(16)]
            s0 = slice(0, CW)
            for k in range(16):
                nc.sync.dma_start(out=wq_sb[:, k, :], in_=d_wq[k])
                nc.sync.dma_start(out=wkv_sb[:, k, :], in_=d_wkv[k])
                nc.sync.dma_start(out=xts[k][:, s0], in_=d_xt[k][:, s0])
            cosq_sb = kconst.tile([128, SEQ], f16, tag="cosq")
            sinq_sb = kconst.tile([128, SEQ], f16, tag="sinq")
            cosk_sb = kconst.tile([64, SEQ], f16, tag="cosk")
            sink_sb = kconst.tile([64, SEQ], f16, tag="sink")
            for c in range(NCH):
                s = slice(CW * c, CW * (c + 1))
                if c > 0:
                    for i in range(16):
                        nc.sync.dma_start(out=xts[i][:, s], in_=d_xt[i][:, s])
                nc.sync.dma_start(out=cosq_sb[:, s], in_=d_cosq[:, s])
                nc.sync.dma_start(out=sinq_sb[:, s], in_=d_sinq[:, s])
                nc.sync.dma_start(out=cosk_sb[:, s], in_=d_cosk[:, s])
                nc.sync.dma_start(out=sink_sb[:, s], in_=d_sink[:, s])
                if c == 0:
                    wo_sb = kconst.tile([128, 2, DIM], f16, tag="wo")
                    nc.sync.dma_start(
                        out=wo_sb, in_=d_wo.ap().rearrange("g p c -> p g c"))

            # persistent state
            qro = [kconst.tile([128, SEQ], f16, tag=f"qro{g}", name=f"qro{g}")
                   for g in range(2)]
            krod = kconst.tile([128, SEQ], f16, tag="krod")
            aot = [kconst.tile([128, SEQ], f16, tag=f"aot{g}", name=f"aot{g}")
                   for g in range(2)]
            vr = [kconst.tile([128, HD], f16, tag=f"vr{j}", name=f"vr{j}")
                  for j in range(16)]
            rden_t = kconst.tile([128, CW], f32, tag="rden")
            nc.vector.memset(rden_t, 1.0)  # rows 64-127 stay benign

            def proj_acc(lhsT_of_k, c):
                pj = aux.tile([128, CW], f32, tag="aux", name=f"pj{c}")
                for k in range(16):
                    nc.tensor.matmul(pj, lhsT_of_k(k),
                                     xts[k][:, CW * c:CW * c + CW],
                                     start=(k == 0), stop=(k == 15))
                return pj

            QUAKE = 0x5F3759DF
            SR = mybir.AluOpType.logical_shift_right
            XOR = mybir.AluOpType.bitwise_xor
            ADDOP = mybir.AluOpType.add
            MULOP = mybir.AluOpType.mult
            MAXOP = mybir.AluOpType.max
            i32 = mybir.dt.int32

            def rsqrt_chain(x, y0, t, out):
                """out := rsqrt(x + 64*EPS) via quake seed + 1 Newton step,
                all on DVE (no ACT tables). x is clobbered."""
                nc.vector.tensor_scalar(out=x, in0=x, scalar1=float(HD) * EPS,
                                        scalar2=1e-12, op0=ADDOP, op1=MAXOP)
                xi = x.bitcast(i32)
                yi = y0.bitcast(i32)
                nc.vector.tensor_scalar(out=yi, in0=xi, scalar1=1,
                                        scalar2=-1, op0=SR, op1=XOR)
                nc.vector.tensor_scalar_add(yi, yi, QUAKE + 1)
                # newton: y = y0 * (1.5 - 0.5 * x * y0^2)
                nc.vector.tensor_mul(t, y0, y0)
                nc.vector.scalar_tensor_tensor(out=t, in0=t, scalar=-0.5,
                                               in1=x, op0=MULOP, op1=MULOP)
                nc.vector.scalar_tensor_tensor(out=out, in0=t, scalar=1.5,
                                               in1=y0, op0=ADDOP, op1=MULOP)

            def q_group(c, g):
                ch = slice(CW * c, CW * (c + 1))
                pj = proj_acc(
                    lambda k: wq_sb[:, k, 128 * g:128 * g + 128], c)
                q16 = work.tile([128, CW], f16, tag="q16", bufs=4,
                                name=f"q16_{c}_{g}")
                nc.vector.tensor_copy(out=q16, in_=pj)
                sqq = work.tile([128, CW], f16, tag="sqq", bufs=2)
                nc.vector.tensor_mul(sqq, q16, q16)
                ssq2 = aux.tile([128, CW], f32, tag="aux", name=f"ssq{c}_{g}")
                nc.tensor.matmul(ssq2, ob_sb, sqq, start=True, stop=True)
                nc.vector.tensor_copy(out=nrm[32 * g:32 * g + 2, :],
                                      in_=ssq2[0:2, :])
                rot = work.tile([128, CW], f16, tag="rot", bufs=2)
                for (o, s_) in ((0, 32), (32, 0), (64, 96), (96, 64)):
                    nc.vector.tensor_copy(out=rot[o:o + 32, :],
                                          in_=q16[s_:s_ + 32, :])
                tmph = work.tile([128, CW], f16, tag="tmph", bufs=2)
                nc.vector.tensor_mul(tmph, rot, sinq_sb[:, ch])
                hhh = work.tile([128, CW], f16, tag="hhh", bufs=2,
                                name=f"hhh{c}_{g}")
                nc.vector.tensor_mul(hhh, q16, cosq_sb[:, ch])
                nc.vector.tensor_add(hhh, hhh, tmph)
                return hhh

            def kv_group(c):
                ch = slice(CW * c, CW * (c + 1))
                pj = proj_acc(lambda k: wkv_sb[:, k, :], c)
                k16 = work.tile([64, CW], f16, tag="k16", bufs=2)
                nc.vector.tensor_copy(out=k16, in_=pj[0:64, :])
                v16 = work.tile([64, CW], f16, tag="v16", bufs=2)
                nc.vector.tensor_copy(out=v16, in_=pj[64:128, :])
                sqk = work.tile([64, CW], f16, tag="sqk", bufs=2)
                nc.vector.tensor_mul(sqk, k16, k16)
                ssk = aux.tile([128, CW], f32, tag="aux", name=f"ssk{c}")
                nc.tensor.matmul(ssk, ob_sb[0:64, :], sqk, start=True, stop=True)
                nc.vector.tensor_copy(out=nrm[64:65, :], in_=ssk[0:1, :])
                # k rope into hk (norm mul after the shared rsqrt chain)
                rotk = work.tile([64, CW], f16, tag="rotk", bufs=2)
                nc.vector.tensor_copy(out=rotk[0:32, :], in_=k16[32:64, :])
                nc.vector.tensor_copy(out=rotk[32:64, :], in_=k16[0:32, :])
                tmpk = work.tile([64, CW], f16, tag="tmpk", bufs=2)
                nc.vector.tensor_mul(tmpk, rotk, sink_sb[:, ch])
                hk = work.tile([64, CW], f16, tag="hk", bufs=2, name=f"hk{c}")
                nc.vector.tensor_mul(hk, k16, cosk_sb[:, ch])
                nc.vector.tensor_add(hk, hk, tmpk)
                # krod written after the shared rsqrt chain (see proj_chunk)
                # v transposes -> vr[4c+b]
                for b in range(4):
                    tp = aux.tile([128, HD], f16, tag="aux", name=f"tp{c}_{b}")
                    nc.tensor.transpose(tp, v16[:, 128 * b:128 * b + 128],
                                        idn_sb)
                    nc.vector.tensor_copy(out=vr[4 * c + b], in_=tp)
                return hk

            def proj_chunk(c):
                """kv, q0, q1 -> one fused rsqrt chain -> norm multiplies."""
                ch = slice(CW * c, CW * (c + 1))
                hk = kv_group(c)
                hhh0 = q_group(c, 0)
                hhh1 = q_group(c, 1)
                # fused chain for rows 0,1 (q0) / 32,33 (q1) / 64 (k)
                rsqrt_chain(nrm[0:65, :], y0t[0:65, :], nrt[0:65, :],
                            nrm[0:65, :])
                nc.vector.tensor_copy(out=nrmb[0:65, :], in_=nrm[0:65, :])
                bk = aux.tile([128, CW], f32, tag="aux", name=f"bk{c}")
                nc.tensor.matmul(bk, selk_sb, nrmb, start=True, stop=True)
                nc.vector.tensor_mul(krod[0:64, ch], hk, bk[0:64, :])
                nc.vector.tensor_copy(out=krod[64:128, ch], in_=krod[0:64, ch])
                for g, hhh in ((0, hhh0), (1, hhh1)):
                    bq = aux.tile([128, CW], f32, tag="aux", name=f"bq{c}_{g}")
                    nc.tensor.matmul(bq, selq_sb[g], nrmb, start=True,
                                     stop=True)
                    nc.vector.tensor_mul(qro[g][:, ch], hhh, bq)

            def scores_exp(c, g):
                nj = 4 * c + 4
                exs = []
                for j in range(nj):
                    p0 = max(0, 128 * j - CW * c)
                    sc = scp.tile([128, 2, CW], f32, tag="sc",
                                  name=f"sc{c}_{g}_{j}")
                    nc.tensor.matmul(
                        sc[:, 0, p0:CW],
                        krod[0:64, 128 * j:128 * j + 128],
                        qro[g][0:64, CW * c + p0:CW * (c + 1)],
                        start=True, stop=True)
                    nc.tensor.matmul(
                        sc[:, 1, p0:CW],
                        krod[64:128, 128 * j:128 * j + 128],
                        qro[g][64:128, CW * c + p0:CW * (c + 1)],
                        start=True, stop=True)
                    ex = expool.tile([128, 2, CW], f16, tag="ex",
                                     name=f"ex{c}_{g}_{j}")
                    nc.scalar.activation(out=ex[:, :, p0:CW],
                                         in_=sc[:, :, p0:CW],
                                         func=EXPF, scale=1.0, bias=0.0)
                    if 128 * j >= CW * c:  # diagonal block
                        for hh in range(2):
                            nc.gpsimd.tensor_mul(
                                ex[:, hh, p0:p0 + 128],
                                ex[:, hh, p0:p0 + 128], tri_sb)
                    exs.append((ex, p0))
                return exs

            def av_den(c, g, exs):
                nj = 4 * c + 4
                ot = otp.tile([128, CW], f32, tag="ot", name=f"ot{c}_{g}")
                for j in range(nj):
                    ex, p0 = exs[j]
                    nc.tensor.matmul(ot[0:64, p0:CW], vr[j],
                                     ex[:, 0, p0:CW],
                                     start=(j == 0), stop=(j == nj - 1))
                    nc.tensor.matmul(ot[64:128, p0:CW], vr[j],
                                     ex[:, 1, p0:CW],
                                     start=(j == 0), stop=(j == nj - 1))
                den = denp.tile([128, CW], f32, tag="den",
                                name=f"den{c}_{g}")
                for j in range(nj):
                    ex, p0 = exs[j]
                    nc.tensor.matmul(den[0:64, p0:CW], one64_sb,
                                     ex[:, 0, p0:CW],
                                     start=(j == 0), stop=(j == nj - 1))
                    nc.tensor.matmul(den[64:128, p0:CW], one64_sb,
                                     ex[:, 1, p0:CW],
                                     start=(j == 0), stop=(j == nj - 1))
                return ot, den

            def norm_out(c, g, ot, den):
                ch = slice(CW * c, CW * (c + 1))
                nc.vector.tensor_scalar_max(rden_t[0:65, :], den[0:65, :],
                                            1e-30)
                nc.vector.reciprocal_approx_fast(out=rden_t[0:65, :],
                                                 in_=rden_t[0:65, :])
                nc.vector.tensor_copy(out=rdenb[0:65, :],
                                      in_=rden_t[0:65, :])
                bs = aux.tile([128, CW], f32, tag="aux", name=f"bs{c}_{g}")
                nc.tensor.matmul(bs, seld_sb, rdenb, start=True, stop=True)
                aotu = work.tile([128, CW], f16, tag="aotu", bufs=2)
                nc.vector.tensor_copy(out=aotu, in_=ot)
                nc.vector.tensor_mul(aot[g][:, ch], aotu, bs)

            def wo_chunk(c):
                for m in range(4 * c, 4 * c + 4):
                    ys = work.tile([128, DIM], f16, tag="ys", bufs=2)
                    for oc in range(4):
                        yp = aux.tile([128, CW], f32, tag="aux",
                                      name=f"yp{m}_{oc}")
                        nc.tensor.matmul(yp, aot[0][:, 128 * m:128 * m + 128],
                                         wo_sb[:, 0, CW * oc:CW * oc + CW],
                                         start=True, stop=False)
                        nc.tensor.matmul(yp, aot[1][:, 128 * m:128 * m + 128],
                                         wo_sb[:, 1, CW * oc:CW * oc + CW],
                                         start=False, stop=True)
                        nc.vector.tensor_copy(
                            out=ys[:, CW * oc:CW * oc + CW], in_=yp)
                    nc.sync.dma_start(out=d_y[m], in_=ys)

            # software-pipelined emission: scores(c) first (feeds ACT asap),
            # then proj(c+1) (long DVE lead time overlaps chunk-c EXP phase),
            # then the elastic PE fill (AV/den/norm/Wo of chunk c).
            proj_chunk(0)
            for c in range(NCH):
                exs0 = scores_exp(c, 0)
                exs1 = scores_exp(c, 1)
                if c + 1 < NCH:
                    proj_chunk(c + 1)
                od0 = av_den(c, 0, exs0)
                od1 = av_den(c, 1, exs1)
                norm_out(c, 0, *od0)
                norm_out(c, 1, *od1)
                wo_chunk(c)
    nc.compile()
    return nc


def _get_nc():
    if "nc" not in _CACHE:
        _maybe_enable_ldw_opt()
        _ensure_ntff_hook()
        _CACHE["nc"] = _build_nc()
    return _CACHE["nc"]


def _make_tables(qn_w, kn_w, start_pos):
    inv = THETA ** (-np.arange(0, HD, 2, dtype=np.float64) / HD)
    pos = float(start_pos) + np.arange(SEQ, dtype=np.float64)
    ang = inv[:, None] * pos[None, :]  # (32, SEQ)
    c, s = np.cos(ang), np.sin(ang)

    def tabs(gain):
        g = gain.astype(np.float64)
        cosg = np.concatenate([g[0:32, None] * c, g[32:64, None] * c], axis=0)
        sing = np.concatenate([-g[32:64, None] * s, g[0:32, None] * s], axis=0)
        return cosg.astype(ml_dtypes.bfloat16), sing.astype(ml_dtypes.bfloat16)

    cq, sq_ = tabs(np.asarray(qn_w))
    ck, sk = tabs(np.asarray(kn_w))
    return (np.ascontiguousarray(np.tile(cq, (2, 1))),
            np.ascontiguousarray(np.tile(sq_, (2, 1))), ck, sk)


def _prep_in_maps(x, Wq, Wk, Wv, Wo, qn_w, kn_w, start_pos):
    bf = ml_dtypes.bfloat16
    xT = np.ascontiguousarray(np.asarray(x)[0].T).astype(bf)
    xt = xT.reshape(16, 128, SEQ)
    cosq, sinq, cosk, sink = _make_tables(qn_w, kn_w, start_pos)
    tri = np.triu(np.ones((128, 128), bf))
    idn = np.eye(64, dtype=bf)
    ob = np.zeros((128, 128), bf)
    ob[0:64, 0] = 1.0
    ob[64:128, 1] = 1.0
    # nrmb rows: 0=q0-even 1=q0-odd 32=q1-even 33=q1-odd 64=k
    selq0 = np.zeros((128, 128), bf)
    selq0[0, 0:64] = 8.0
    selq0[1, 64:128] = 8.0
    selq1 = np.zeros((128, 128), bf)
    selq1[32, 0:64] = 8.0
    selq1[33, 64:128] = 8.0
    selk = np.zeros((128, 128), bf)
    selk[64, 0:64] = 1.0
    seld = np.zeros((128, 128), bf)
    seld[0, 0:64] = 1.0
    seld[64, 64:128] = 1.0
    one64 = np.zeros((128, 64), bf)
    one64[:, 0] = 1.0
    Wq, Wk, Wv, Wo = (np.asarray(a) for a in (Wq, Wk, Wv, Wo))
    in_maps = []
    for cid in range(NCORES):
        wq_c = np.ascontiguousarray(
            Wq[:, 256 * cid:256 * (cid + 1)]).astype(bf).reshape(16, 128, 256)
        wkv_c = np.ascontiguousarray(np.concatenate(
            [Wk[:, HD * cid:HD * (cid + 1)], Wv[:, HD * cid:HD * (cid + 1)]],
            axis=1)).astype(bf).reshape(16, 128, 128)
        wo_c = np.ascontiguousarray(
            Wo[256 * cid:256 * (cid + 1), :]).astype(bf).reshape(2, 128, DIM)
        in_maps.append({"xt": xt, "wq": wq_c, "wkv": wkv_c, "wo": wo_c,
                        "cosq": cosq, "sinq": sinq, "cosk": cosk, "sink": sink,
                        "tri": tri, "idn": idn, "ob": ob, "selq0": selq0,
                        "selq1": selq1, "selk": selk, "seld": seld,
                        "one64": one64})
    return in_maps


def run(inputs, trace=False, **kw):
    from concourse import bass_utils
    nc = _get_nc()
    in_maps = _prep_in_maps(
        inputs["x"], inputs["Wq"], inputs["Wk"], inputs["Wv"], inputs["Wo"],
        inputs["qn_w"], inputs["kn_w"], inputs["start_pos"])
    res = bass_utils.run_bass_kernel_spmd(
        nc, in_maps, core_ids=list(range(NCORES)), trace=trace, **kw)
    y = np.zeros((SEQ, DIM), np.float32)
    for r in res.results:
        y += r["y"].reshape(SEQ, DIM).astype(np.float32)
    return y.reshape(1, SEQ, DIM), res


def kernel(x, Wq, Wk, Wv, Wo, qn_w, kn_w, mask, start_pos):
    out, _ = run(dict(x=x, Wq=Wq, Wk=Wk, Wv=Wv, Wo=Wo, qn_w=qn_w, kn_w=kn_w,
                      mask=mask, start_pos=start_pos))
    return out


# revision 3
# speedup vs baseline: 1.0766x; 1.0766x over previous
"""GQA kernel for trn2, 8 NeuronCores, tensor-parallel over heads.

Sharding: 4 q heads + 1 kv head per core (column-split Wq/Wk/Wv, row-split
Wo), partial outputs summed on host. bf16 matmuls, fp32 PSUM.

Design:
  - software-pipelined over 4 qs-chunks of 512: per chunk, scores+exp are
    emitted first (feed the ACT engine asap), then the NEXT chunk's
    projections (their long DVE chain overlaps this chunk's softmax),
    then AV/den/normalization, then the output projection.
  - DMA transfer count minimized (descriptor generation is ~600ns serial
    on the Sync engine): x loads as chunk-0 slices first, then the
    remainder, so the first projections start ~6us in.
  - HAM warmup matmuls keep the PE at 2.4GHz through the DMA window.
  - score matmuls for a head PAIR run concurrently via 64x128 row tiling
    (heads at SBUF partitions 0-63/64-127, kro duplicated on both halves,
    outputs in the two banks of one [128,2,512] PSUM tile); one EXP
    instruction covers both heads via a strided AP.
  - AV col-packed via 128x64 tiling (shared vr lhsT for tiles (0,0)/(0,64));
    softmax denominators via ones-column lhsT matmuls in the same mode,
    accumulated per head in rows 0/64 of one PSUM bank.
  - all rsqrts (QK-norm) computed on DVE with a quake-seed + Newton step
    (int ALU on bitcast fp32), so the ACT table set never leaves
    exp_and_others: zero table reloads in steady state.
  - q-norm gains broadcast via selector matmuls; softmax 1/den via DVE
    reciprocal_approx_fast + selector matmul; causal tri masking on GpSimd.
"""

import sys
import types
import numpy as np
import ml_dtypes

for _p in ("/opt/trn_rl_repo",):
    if _p not in sys.path:
        sys.path.append(_p)

SEQ = 2048
DIM = 2048
HD = 64
NCORES = 8
EPS = 1e-6
THETA = 10000.0
NCH = 4          # qs chunks
CW = 512         # chunk width

_CACHE = {}


def _maybe_enable_ldw_opt():
    """Experiment: walrus is invoked with --enable-ldw-opt=false by default;
    flip it via env BASS_LDW_OPT=1 to measure the LDWEIGHTS pipelining win."""
    import os
    if os.environ.get("BASS_LDW_OPT") != "1":
        return
    from concourse import bass_utils as bu
    if getattr(bu, "_ldw_patched", False):
        return
    orig = bu.run_command

    def run_command(cmd, *a, **kw):
        if isinstance(cmd, list):
            cmd = ["--enable-ldw-opt=true" if c == "--enable-ldw-opt=false"
                   else c for c in cmd]
        return orig(cmd, *a, **kw)

    bu.run_command = run_command
    bu._ldw_patched = True


def _ensure_ntff_hook():
    if "antenv.axon_hooks" in sys.modules:
        return
    try:
        import antenv
        m = types.ModuleType("antenv.axon_hooks")
        hook = [None]
        m.set_axon_ntff_profile_hook = lambda h: hook.__setitem__(0, h)
        m.get_axon_ntff_profile_hook = lambda: hook[0]
        sys.modules["antenv.axon_hooks"] = m
        antenv.axon_hooks = m
        from trn_agent_boot.trn_boot import _ntff_profile_via_ctypes
        m.set_axon_ntff_profile_hook(
            _ntff_profile_via_ctypes("/opt/axon/libaxon_pjrt.so"))
    except Exception:
        pass


def _build_nc():
    import concourse.mybir as mybir
    import concourse.tile as tile
    from concourse import bacc

    f32, f16 = mybir.dt.float32, mybir.dt.bfloat16
    EXPF = mybir.ActivationFunctionType.Exp
    SQF = mybir.ActivationFunctionType.Square

    nc = bacc.Bacc("TRN2", target_bir_lowering=False, debug=False,
                   num_devices=NCORES)

    d_xt = nc.dram_tensor("xt", (16, 128, SEQ), f16, kind="ExternalInput")
    d_wq = nc.dram_tensor("wq", (16, 128, 256), f16, kind="ExternalInput")
    d_wkv = nc.dram_tensor("wkv", (16, 128, 128), f16, kind="ExternalInput")
    d_wo = nc.dram_tensor("wo", (2, 128, DIM), f16, kind="ExternalInput")
    d_cosq = nc.dram_tensor("cosq", (128, SEQ), f16, kind="ExternalInput")
    d_sinq = nc.dram_tensor("sinq", (128, SEQ), f16, kind="ExternalInput")
    d_cosk = nc.dram_tensor("cosk", (64, SEQ), f16, kind="ExternalInput")
    d_sink = nc.dram_tensor("sink", (64, SEQ), f16, kind="ExternalInput")
    d_tri = nc.dram_tensor("tri", (128, 128), f16, kind="ExternalInput")
    d_idn = nc.dram_tensor("idn", (64, 64), f16, kind="ExternalInput")
    d_ob = nc.dram_tensor("ob", (128, 128), f16, kind="ExternalInput")
    d_selq0 = nc.dram_tensor("selq0", (128, 128), f16, kind="ExternalInput")
    d_selq1 = nc.dram_tensor("selq1", (128, 128), f16, kind="ExternalInput")
    d_selk = nc.dram_tensor("selk", (128, 128), f16, kind="ExternalInput")
    d_seld = nc.dram_tensor("seld", (128, 128), f16, kind="ExternalInput")
    d_one64 = nc.dram_tensor("one64", (128, 64), f16, kind="ExternalInput")
    d_y = nc.dram_tensor("y", (16, 128, DIM), f16, kind="ExternalOutput")

    with tile.TileContext(nc) as tc:
        from contextlib import ExitStack
        with ExitStack() as ctx:
            kconst = ctx.enter_context(tc.tile_pool(name="kconst", bufs=1))
            xpool = ctx.enter_context(tc.tile_pool(name="xp", bufs=1))
            work = ctx.enter_context(tc.tile_pool(name="work", bufs=1))
            expool = ctx.enter_context(tc.tile_pool(name="ep", bufs=18))
            scp = ctx.enter_context(
                tc.tile_pool(name="scp", bufs=1, space="PSUM"))
            otp = ctx.enter_context(
                tc.tile_pool(name="otp", bufs=2, space="PSUM"))
            denp = ctx.enter_context(
                tc.tile_pool(name="denp", bufs=2, space="PSUM"))
            aux = ctx.enter_context(
                tc.tile_pool(name="aux", bufs=2, space="PSUM"))

            # ---- small consts first (cheap DMAs) --------------------------
            tri_sb = kconst.tile([128, 128], f16, tag="tri")
            nc.sync.dma_start(out=tri_sb, in_=d_tri[:, :])
            idn_sb = kconst.tile([64, 64], f16, tag="idn")
            nc.sync.dma_start(out=idn_sb, in_=d_idn[:, :])
            ob_sb = kconst.tile([128, 128], f16, tag="ob")
            nc.sync.dma_start(out=ob_sb, in_=d_ob[:, :])
            selq_sb = [kconst.tile([128, 128], f16, tag=f"selq{g}",
                                   name=f"selq{g}") for g in range(2)]
            nc.sync.dma_start(out=selq_sb[0], in_=d_selq0[:, :])
            nc.sync.dma_start(out=selq_sb[1], in_=d_selq1[:, :])
            selk_sb = kconst.tile([128, 128], f16, tag="selk")
            nc.sync.dma_start(out=selk_sb, in_=d_selk[:, :])
            seld_sb = kconst.tile([128, 128], f16, tag="seld")
            nc.sync.dma_start(out=seld_sb, in_=d_seld[:, :])
            one64_sb = kconst.tile([128, 64], f16, tag="one64")
            nc.sync.dma_start(out=one64_sb, in_=d_one64[:, :])

            # HAM warmup: keep the PE busy (and at 2.4GHz) through the
            # initial x/weight DMA window so the first projections run warm.
            warm = aux.tile([128, 128], f32, tag="aux", name="warm")
            for _w in range(64):
                nc.tensor.matmul(warm, tri_sb, tri_sb, start=True, stop=True)

            # rsqrt scratch (rows 0-4 live; rest zeroed once)
            nrm = kconst.tile([128, CW], f32, tag="nrm")
            nc.vector.memset(nrm, 1.0)
            nrmb = kconst.tile([128, CW], f16, tag="nrmb")
            nc.vector.memset(nrmb, 0.0)
            y0t = kconst.tile([72, CW], f32, tag="y0t")
            nrt = kconst.tile([72, CW], f32, tag="nrt")
            rdenb = kconst.tile([128, CW], f16, tag="rdenb")
            nc.vector.memset(rdenb, 0.0)

            # few, large DMAs: descriptor generation on the Sync engine is
            # ~600ns per dma_start, so minimize transfer count.
            wkv_sb = kconst.tile([128, 16, 128], f16, tag="wkv")
            nc.sync.dma_start(out=wkv_sb,
                              in_=d_wkv.ap().rearrange("i p c -> p i c"))
            wq_sb = kconst.tile([128, 16, 256], f16, tag="wq")
            nc.sync.dma_start(out=wq_sb,
                              in_=d_wq.ap().rearrange("i p c -> p i c"))
            xt_all = xpool.tile([128, 16, SEQ], f16, tag="xt")
            for i in range(16):
                nc.sync.dma_start(out=xt_all[:, i, 0:CW],
                                  in_=d_xt[i][:, 0:CW])
            cosq_sb = kconst.tile([128, SEQ], f16, tag="cosq")
            nc.sync.dma_start(out=cosq_sb, in_=d_cosq[:, :])
            sinq_sb = kconst.tile([128, SEQ], f16, tag="sinq")
            nc.sync.dma_start(out=sinq_sb, in_=d_sinq[:, :])
            cosk_sb = kconst.tile([64, SEQ], f16, tag="cosk")
            nc.sync.dma_start(out=cosk_sb, in_=d_cosk[:, :])
            sink_sb = kconst.tile([64, SEQ], f16, tag="sink")
            nc.sync.dma_start(out=sink_sb, in_=d_sink[:, :])
            for i in range(16):
                nc.sync.dma_start(out=xt_all[:, i, CW:SEQ],
                                  in_=d_xt[i][:, CW:SEQ])
                if i == 0:
                    wo_sb = kconst.tile([128, 2, DIM], f16, tag="wo")
                    nc.sync.dma_start(
                        out=wo_sb, in_=d_wo.ap().rearrange("g p c -> p g c"))

            # persistent state
            qro = [kconst.tile([128, SEQ], f16, tag=f"qro{g}", name=f"qro{g}")
                   for g in range(2)]
            krod = kconst.tile([128, SEQ], f16, tag="krod")
            aot = [kconst.tile([128, SEQ], f16, tag=f"aot{g}", name=f"aot{g}")
                   for g in range(2)]
            vr = [kconst.tile([128, HD], f16, tag=f"vr{j}", name=f"vr{j}")
                  for j in range(16)]
            rden_t = kconst.tile([128, CW], f32, tag="rden")
            nc.vector.memset(rden_t, 1.0)  # rows 64-127 stay benign

            def proj_acc(lhsT_of_k, c):
                pj = aux.tile([128, CW], f32, tag="aux", name=f"pj{c}")
                for k in range(16):
                    nc.tensor.matmul(pj, lhsT_of_k(k),
                                     xt_all[:, k, CW * c:CW * c + CW],
                                     start=(k == 0), stop=(k == 15))
                return pj

            QUAKE = 0x5F3759DF
            SR = mybir.AluOpType.logical_shift_right
            XOR = mybir.AluOpType.bitwise_xor
            ADDOP = mybir.AluOpType.add
            MULOP = mybir.AluOpType.mult
            MAXOP = mybir.AluOpType.max
            i32 = mybir.dt.int32

            def rsqrt_chain(x, y0, t, out):
                """out := rsqrt(x + 64*EPS) via quake seed + 1 Newton step,
                all on DVE (no ACT tables). x is clobbered."""
                nc.vector.tensor_scalar(out=x, in0=x, scalar1=float(HD) * EPS,
                                        scalar2=1e-12, op0=ADDOP, op1=MAXOP)
                xi = x.bitcast(i32)
                yi = y0.bitcast(i32)
                nc.vector.tensor_scalar(out=yi, in0=xi, scalar1=1,
                                        scalar2=-1, op0=SR, op1=XOR)
                nc.vector.tensor_scalar_add(yi, yi, QUAKE + 1)
                # newton: y = y0 * (1.5 - 0.5 * x * y0^2)
                nc.vector.tensor_mul(t, y0, y0)
                nc.vector.scalar_tensor_tensor(out=t, in0=t, scalar=-0.5,
                                               in1=x, op0=MULOP, op1=MULOP)
                nc.vector.scalar_tensor_tensor(out=out, in0=t, scalar=1.5,
                                               in1=y0, op0=ADDOP, op1=MULOP)

            def q_group(c, g):
                ch = slice(CW * c, CW * (c + 1))
                pj = proj_acc(
                    lambda k: wq_sb[:, k, 128 * g:128 * g + 128], c)
                q16 = work.tile([128, CW], f16, tag="q16", bufs=4,
                                name=f"q16_{c}_{g}")
                nc.vector.tensor_copy(out=q16, in_=pj)
                sqq = work.tile([128, CW], f16, tag="sqq", bufs=2)
                nc.vector.tensor_mul(sqq, q16, q16)
                ssq2 = aux.tile([128, CW], f32, tag="aux", name=f"ssq{c}_{g}")
                nc.tensor.matmul(ssq2, ob_sb, sqq, start=True, stop=True)
                nc.vector.tensor_copy(out=nrm[32 * g:32 * g + 2, :],
                                      in_=ssq2[0:2, :])
                rot = work.tile([128, CW], f16, tag="rot", bufs=2)
                for (o, s_) in ((0, 32), (32, 0), (64, 96), (96, 64)):
                    nc.vector.tensor_copy(out=rot[o:o + 32, :],
                                          in_=q16[s_:s_ + 32, :])
                tmph = work.tile([128, CW], f16, tag="tmph", bufs=2)
                nc.vector.tensor_mul(tmph, rot, sinq_sb[:, ch])
                hhh = work.tile([128, CW], f16, tag="hhh", bufs=2,
                                name=f"hhh{c}_{g}")
                nc.vector.tensor_mul(hhh, q16, cosq_sb[:, ch])
                nc.vector.tensor_add(hhh, hhh, tmph)
                return hhh

            def kv_group(c):
                ch = slice(CW * c, CW * (c + 1))
                pj = proj_acc(lambda k: wkv_sb[:, k, :], c)
                k16 = work.tile([64, CW], f16, tag="k16", bufs=2)
                nc.vector.tensor_copy(out=k16, in_=pj[0:64, :])
                v16 = work.tile([64, CW], f16, tag="v16", bufs=2)
                nc.vector.tensor_copy(out=v16, in_=pj[64:128, :])
                sqk = work.tile([64, CW], f16, tag="sqk", bufs=2)
                nc.vector.tensor_mul(sqk, k16, k16)
                ssk = aux.tile([128, CW], f32, tag="aux", name=f"ssk{c}")
                nc.tensor.matmul(ssk, ob_sb[0:64, :], sqk, start=True, stop=True)
                nc.vector.tensor_copy(out=nrm[64:65, :], in_=ssk[0:1, :])
                # k rope into hk (norm mul after the shared rsqrt chain)
                rotk = work.tile([64, CW], f16, tag="rotk", bufs=2)
                nc.vector.tensor_copy(out=rotk[0:32, :], in_=k16[32:64, :])
                nc.vector.tensor_copy(out=rotk[32:64, :], in_=k16[0:32, :])
                tmpk = work.tile([64, CW], f16, tag="tmpk", bufs=2)
                nc.vector.tensor_mul(tmpk, rotk, sink_sb[:, ch])
                hk = work.tile([64, CW], f16, tag="hk", bufs=2, name=f"hk{c}")
                nc.vector.tensor_mul(hk, k16, cosk_sb[:, ch])
                nc.vector.tensor_add(hk, hk, tmpk)
                # krod written after the shared rsqrt chain (see proj_chunk)
                # v transposes -> vr[4c+b]
                for b in range(4):
                    tp = aux.tile([128, HD], f16, tag="aux", name=f"tp{c}_{b}")
                    nc.tensor.transpose(tp, v16[:, 128 * b:128 * b + 128],
                                        idn_sb)
                    nc.vector.tensor_copy(out=vr[4 * c + b], in_=tp)
                return hk

            def proj_chunk(c):
                """kv, q0, q1 -> one fused rsqrt chain -> norm multiplies."""
                ch = slice(CW * c, CW * (c + 1))
                hk = kv_group(c)
                hhh0 = q_group(c, 0)
                hhh1 = q_group(c, 1)
                # fused chain for rows 0,1 (q0) / 32,33 (q1) / 64 (k)
                rsqrt_chain(nrm[0:65, :], y0t[0:65, :], nrt[0:65, :],
                            nrm[0:65, :])
                nc.vector.tensor_copy(out=nrmb[0:65, :], in_=nrm[0:65, :])
                bk = aux.tile([128, CW], f32, tag="aux", name=f"bk{c}")
                nc.tensor.matmul(bk, selk_sb, nrmb, start=True, stop=True)
                nc.vector.tensor_mul(krod[0:64, ch], hk, bk[0:64, :])
                nc.vector.tensor_copy(out=krod[64:128, ch], in_=krod[0:64, ch])
                for g, hhh in ((0, hhh0), (1, hhh1)):
                    bq = aux.tile([128, CW], f32, tag="aux", name=f"bq{c}_{g}")
                    nc.tensor.matmul(bq, selq_sb[g], nrmb, start=True,
                                     stop=True)
                    nc.vector.tensor_mul(qro[g][:, ch], hhh, bq)

            def scores_exp(c, g):
                nj = 4 * c + 4
                exs = []
                for j in range(nj):
                    p0 = max(0, 128 * j - CW * c)
                    sc = scp.tile([128, 2, CW], f32, tag="sc",
                                  name=f"sc{c}_{g}_{j}")
                    nc.tensor.matmul(
                        sc[:, 0, p0:CW],
                        krod[0:64, 128 * j:128 * j + 128],
                        qro[g][0:64, CW * c + p0:CW * (c + 1)],
                        start=True, stop=True)
                    nc.tensor.matmul(
                        sc[:, 1, p0:CW],
                        krod[64:128, 128 * j:128 * j + 128],
                        qro[g][64:128, CW * c + p0:CW * (c + 1)],
                        start=True, stop=True)
                    ex = expool.tile([128, 2, CW], f16, tag="ex",
                                     name=f"ex{c}_{g}_{j}")
                    nc.scalar.activation(out=ex[:, :, p0:CW],
                                         in_=sc[:, :, p0:CW],
                                         func=EXPF, scale=1.0, bias=0.0)
                    if 128 * j >= CW * c:  # diagonal block
                        for hh in range(2):
                            nc.gpsimd.tensor_mul(
                                ex[:, hh, p0:p0 + 128],
                                ex[:, hh, p0:p0 + 128], tri_sb)
                    exs.append((ex, p0))
                return exs

            def av_den(c, g, exs):
                nj = 4 * c + 4
                ot = otp.tile([128, CW], f32, tag="ot", name=f"ot{c}_{g}")
                for j in range(nj):
                    ex, p0 = exs[j]
                    nc.tensor.matmul(ot[0:64, p0:CW], vr[j],
                                     ex[:, 0, p0:CW],
                                     start=(j == 0), stop=(j == nj - 1))
                    nc.tensor.matmul(ot[64:128, p0:CW], vr[j],
                                     ex[:, 1, p0:CW],
                                     start=(j == 0), stop=(j == nj - 1))
                den = denp.tile([128, CW], f32, tag="den",
                                name=f"den{c}_{g}")
                for j in range(nj):
                    ex, p0 = exs[j]
                    nc.tensor.matmul(den[0:64, p0:CW], one64_sb,
                                     ex[:, 0, p0:CW],
                                     start=(j == 0), stop=(j == nj - 1))
                    nc.tensor.matmul(den[64:128, p0:CW], one64_sb,
                                     ex[:, 1, p0:CW],
                                     start=(j == 0), stop=(j == nj - 1))
                return ot, den

            def norm_out(c, g, ot, den):
                ch = slice(CW * c, CW * (c + 1))
                nc.vector.tensor_scalar_max(rden_t[0:65, :], den[0:65, :],
                                            1e-30)
                nc.vector.reciprocal_approx_fast(out=rden_t[0:65, :],
                                                 in_=rden_t[0:65, :])
                nc.vector.tensor_copy(out=rdenb[0:65, :],
                                      in_=rden_t[0:65, :])
                bs = aux.tile([128, CW], f32, tag="aux", name=f"bs{c}_{g}")
                nc.tensor.matmul(bs, seld_sb, rdenb, start=True, stop=True)
                aotu = work.tile([128, CW], f16, tag="aotu", bufs=2)
                nc.vector.tensor_copy(out=aotu, in_=ot)
                nc.vector.tensor_mul(aot[g][:, ch], aotu, bs)

            def wo_chunk(c):
                for m in range(4 * c, 4 * c + 4):
                    ys = work.tile([128, DIM], f16, tag="ys", bufs=2)
                    for oc in range(4):
                        yp = aux.tile([128, CW], f32, tag="aux",
                                      name=f"yp{m}_{oc}")
                        nc.tensor.matmul(yp, aot[0][:, 128 * m:128 * m + 128],
                                         wo_sb[:, 0, CW * oc:CW * oc + CW],
                                         start=True, stop=False)
                        nc.tensor.matmul(yp, aot[1][:, 128 * m:128 * m + 128],
                                         wo_sb[:, 1, CW * oc:CW * oc + CW],
                                         start=False, stop=True)
                        nc.vector.tensor_copy(
                            out=ys[:, CW * oc:CW * oc + CW], in_=yp)
                    nc.sync.dma_start(out=d_y[m], in_=ys)

            # software-pipelined emission: scores(c) first (feeds ACT asap),
            # then proj(c+1) (long DVE lead time overlaps chunk-c EXP phase),
            # then the elastic PE fill (AV/den/norm/Wo of chunk c).
            proj_chunk(0)
            for c in range(NCH):
                exs0 = scores_exp(c, 0)
                exs1 = scores_exp(c, 1)
                if c + 1 < NCH:
                    proj_chunk(c + 1)
                od0 = av_den(c, 0, exs0)
                od1 = av_den(c, 1, exs1)
                norm_out(c, 0, *od0)
                norm_out(c, 1, *od1)
                wo_chunk(c)
    nc.compile()
    return nc


def _get_nc():
    if "nc" not in _CACHE:
        _maybe_enable_ldw_opt()
        _ensure_ntff_hook()
        _CACHE["nc"] = _build_nc()
    return _CACHE["nc"]


def _make_tables(qn_w, kn_w, start_pos):
    inv = THETA ** (-np.arange(0, HD, 2, dtype=np.float64) / HD)
    pos = float(start_pos) + np.arange(SEQ, dtype=np.float64)
    ang = inv[:, None] * pos[None, :]  # (32, SEQ)
    c, s = np.cos(ang), np.sin(ang)

    def tabs(gain):
        g = gain.astype(np.float64)
        cosg = np.concatenate([g[0:32, None] * c, g[32:64, None] * c], axis=0)
        sing = np.concatenate([-g[32:64, None] * s, g[0:32, None] * s], axis=0)
        return cosg.astype(ml_dtypes.bfloat16), sing.astype(ml_dtypes.bfloat16)

    cq, sq_ = tabs(np.asarray(qn_w))
    ck, sk = tabs(np.asarray(kn_w))
    return (np.ascontiguousarray(np.tile(cq, (2, 1))),
            np.ascontiguousarray(np.tile(sq_, (2, 1))), ck, sk)


def _prep_in_maps(x, Wq, Wk, Wv, Wo, qn_w, kn_w, start_pos):
    bf = ml_dtypes.bfloat16
    xT = np.ascontiguousarray(np.asarray(x)[0].T).astype(bf)
    xt = xT.reshape(16, 128, SEQ)
    cosq, sinq, cosk, sink = _make_tables(qn_w, kn_w, start_pos)
    tri = np.triu(np.ones((128, 128), bf))
    idn = np.eye(64, dtype=bf)
    ob = np.zeros((128, 128), bf)
    ob[0:64, 0] = 1.0
    ob[64:128, 1] = 1.0
    # nrmb rows: 0=q0-even 1=q0-odd 32=q1-even 33=q1-odd 64=k
    selq0 = np.zeros((128, 128), bf)
    selq0[0, 0:64] = 8.0
    selq0[1, 64:128] = 8.0
    selq1 = np.zeros((128, 128), bf)
    selq1[32, 0:64] = 8.0
    selq1[33, 64:128] = 8.0
    selk = np.zeros((128, 128), bf)
    selk[64, 0:64] = 1.0
    seld = np.zeros((128, 128), bf)
    seld[0, 0:64] = 1.0
    seld[64, 64:128] = 1.0
    one64 = np.zeros((128, 64), bf)
    one64[:, 0] = 1.0
    Wq, Wk, Wv, Wo = (np.asarray(a) for a in (Wq, Wk, Wv, Wo))
    in_maps = []
    for cid in range(NCORES):
        wq_c = np.ascontiguousarray(
            Wq[:, 256 * cid:256 * (cid + 1)]).astype(bf).reshape(16, 128, 256)
        wkv_c = np.ascontiguousarray(np.concatenate(
            [Wk[:, HD * cid:HD * (cid + 1)], Wv[:, HD * cid:HD * (cid + 1)]],
            axis=1)).astype(bf).reshape(16, 128, 128)
        wo_c = np.ascontiguousarray(
            Wo[256 * cid:256 * (cid + 1), :]).astype(bf).reshape(2, 128, DIM)
        in_maps.append({"xt": xt, "wq": wq_c, "wkv": wkv_c, "wo": wo_c,
                        "cosq": cosq, "sinq": sinq, "cosk": cosk, "sink": sink,
                        "tri": tri, "idn": idn, "ob": ob, "selq0": selq0,
                        "selq1": selq1, "selk": selk, "seld": seld,
                        "one64": one64})
    return in_maps


def run(inputs, trace=False, **kw):
    from concourse import bass_utils
    nc = _get_nc()
    in_maps = _prep_in_maps(
        inputs["x"], inputs["Wq"], inputs["Wk"], inputs["Wv"], inputs["Wo"],
        inputs["qn_w"], inputs["kn_w"], inputs["start_pos"])
    res = bass_utils.run_bass_kernel_spmd(
        nc, in_maps, core_ids=list(range(NCORES)), trace=trace, **kw)
    y = np.zeros((SEQ, DIM), np.float32)
    for r in res.results:
        y += r["y"].reshape(SEQ, DIM).astype(np.float32)
    return y.reshape(1, SEQ, DIM), res


def kernel(x, Wq, Wk, Wv, Wo, qn_w, kn_w, mask, start_pos):
    out, _ = run(dict(x=x, Wq=Wq, Wk=Wk, Wv=Wv, Wo=Wo, qn_w=qn_w, kn_w=kn_w,
                      mask=mask, start_pos=start_pos))
    return out
